# revision 23
# baseline (speedup 1.0000x reference)
"""Block-quantized FP8 linear (KLinearFP8) on 8 trn2 NeuronCores.

y[m, n] = sum_k x_dq[m, k] * w_dq[n, k]
  x_dq: per-(row, 128-block) fp8e4m3fn-simulated quantization of x
  w_dq: weight (fp8 values held in fp32) * per-128x128-block scale

Sharding: column-parallel. weight/weight_scale_inv split along N across 8
cores, x replicated; each core computes y[:, c*2048:(c+1)*2048].

Per-core kernel: dequantize both operands to bf16 on-chip (TRN e4m3 max is
240 vs OCP's 448, so x is quantized with scale amax/224 — a power-of-two
rescale of the reference's amax/448 grid, giving identical rounding), then
a k-on-partitions bf16 GEMM with fp32 PSUM accumulation; output in bf16.

Engine layout (from HW traces): PE runs only GEMM matmuls (~884us floor).
All XBAR transposes share the sync ring (two rings corrupt data). Weight
quarters load as fp32 on the scalar HWDGE ring (SWDGE/gpsimd issuing has
multi-us hidden overheads), scale+cast mults alternate vector/gpsimd,
x quant chains run on vector (m-tile 1 on gpsimd to parallelize the
startup). A snake schedule over the first 4 m-tiles keeps the matmul
stream dense (HAM warm) while the 32MB weight read streams in.
"""

import numpy as np

M, K, N = 4096, 4096, 16384
NCORES = 8
NSH = N // NCORES          # 2048 columns of y per core
P = 128
KB = K // P                # 32 k-blocks
NB = NSH // P              # 16 n-blocks per core
FP8_SAFE = 224.0           # 448/2: fits TRN e4m3 (max 240), same rounding grid

_NC_CACHE = {}


def _build(M=M, K=K, NSH=NSH, debug=False):
    import concourse.bass as bass  # noqa: F401
    import concourse.mybir as mybir
    import concourse.tile as tile
    from concourse import bacc

    KB = K // P                # k-blocks
    KH = KB // 2               # k-blocks per half
    KQ = max(KB // 4, 1)       # k-blocks per quarter (w pieces)
    NQ = KB // KQ              # quarters per full K
    MT = M // P                # m-tiles
    NB = NSH // P              # n-blocks
    CHW = min(512, NSH)        # psum chunk width
    NCH = NSH // CHW           # chunks per core
    NPC = CHW // P             # n-blocks per chunk

    f32, bf16, f8 = mybir.dt.float32, mybir.dt.bfloat16, mybir.dt.float8e4

    nc = bacc.Bacc(None, target_bir_lowering=False, debug=debug)
    x_d = nc.declare_dram_parameter("x", [M, K], f32, isOutput=False)
    w_d = nc.declare_dram_parameter("w", [NSH, K], f32, isOutput=False)
    ws_d = nc.declare_dram_parameter("ws", [NB, KB], f32, isOutput=False)
    y_d = nc.declare_dram_parameter("y", [M, NSH], bf16, isOutput=True)

    with tile.TileContext(nc) as tc:
        with (
            tc.tile_pool(name="const", bufs=1) as const,
            tc.tile_pool(name="wt", bufs=1) as wtp,
            tc.tile_pool(name="wdq", bufs=2) as wpool,
            tc.tile_pool(name="wout", bufs=3) as wopool,
            tc.tile_pool(name="xrow", bufs=2) as xpool,
            tc.tile_pool(name="xq", bufs=2) as xqp,
            tc.tile_pool(name="xdq", bufs=2) as xdp,
            tc.tile_pool(name="xt", bufs=8) as xtp,
            tc.tile_pool(name="scales", bufs=4) as spool,
            tc.tile_pool(name="ypool", bufs=1) as ypool,
            tc.tile_pool(name="psum", bufs=8, space="PSUM") as psum,
        ):
            # ---- weight-block scales, broadcast to all partitions ----
            ws_row = const.tile([1, NB * KB], f32)
            nc.scalar.dma_start(
                ws_row[:], ws_d[:].rearrange("a b -> (a b)")[None, :]
            )
            ws_b = const.tile([P, NB, KB], f32)
            nc.gpsimd.partition_broadcast(
                ws_b[:].rearrange("p a b -> p (a b)"), ws_row[:]
            )

            # Transposed weights, one tile per chunk: [k-part, nbL, kb, n].
            # Chunk c's matmuls stream wTc[c][:, :, kb, :] (3D strided AP);
            # each XBAR destination [:, nbL, kq-range, :] is contiguous.
            wTc = [
                wtp.tile([P, NPC, KB, P], bf16, name=f"wT{c}")
                for c in range(NCH)
            ]

            wqi = [0]

            def w_piece(nb, kq):
                # fp32 quarter on the scalar HWDGE ring; scale+cast to bf16
                # on vector/gpsimd (alternating); transpose on sync XBAR.
                ks = slice(kq * KQ * P, (kq + 1) * KQ * P)
                wdq = wpool.tile([P, KQ, P], f32, tag="wdq")
                nc.scalar.dma_start(
                    wdq[:],
                    w_d[nb * P:(nb + 1) * P, ks].rearrange(
                        "n (kb x) -> n kb x", x=P
                    ),
                )
                wdb = wopool.tile([P, KQ, P], bf16, tag="wout")
                eng = nc.vector if wqi[0] % 2 == 0 else nc.gpsimd
                wqi[0] += 1
                eng.tensor_tensor(
                    wdb[:], wdq[:],
                    ws_b[:, nb, kq * KQ:(kq + 1) * KQ, None].to_broadcast(
                        (P, KQ, P)
                    ),
                    mybir.AluOpType.mult,
                )
                c, nbL = nb // NPC, nb % NPC
                nc.sync.dma_start_transpose(
                    wTc[c][:, nbL, kq * KQ:(kq + 1) * KQ, :],
                    wdb[:].rearrange("p a b -> p (a b)"),
                )

            def w_group(c, kq):
                for nb in range(c * NPC, (c + 1) * NPC):
                    w_piece(nb, kq)

            xts = {}

            def x_prep(mt):
                # quantize+dequantize one m-tile of x, half-K at a time, onto
                # the reference fp8 grid; XBAR-transpose to k-on-partitions.
                # m-tile 1 runs on gpsimd so its chain overlaps m-tile 0's.
                ms = slice(mt * P, (mt + 1) * P)
                eng = nc.gpsimd if mt == 1 else nc.vector
                halves = []
                for kh in range(2):
                    ks = slice(kh * KH * P, (kh + 1) * KH * P)
                    xrow = xpool.tile([P, KH, P], f32, tag="xrow")
                    nc.scalar.dma_start(
                        xrow[:],
                        x_d[ms, ks].rearrange("m (kb x) -> m kb x", x=P),
                    )
                    sc = spool.tile([P, 3, KH], f32, tag="sc")
                    amax, rinv, s2 = sc[:, 0, :], sc[:, 1, :], sc[:, 2, :]
                    # free-axis reduce is vector-only
                    nc.vector.tensor_reduce(
                        amax, xrow[:], axis=mybir.AxisListType.X,
                        op=mybir.AluOpType.max, apply_absolute_value=True,
                    )
                    nc.vector.reciprocal(rinv, amax)
                    eng.tensor_scalar_mul(rinv, rinv, float(FP8_SAFE))
                    eng.tensor_scalar_mul(s2, amax, float(1.0 / FP8_SAFE))
                    xq = xqp.tile([P, KH, P], f8, tag="xq")
                    eng.tensor_tensor(
                        xq[:], xrow[:],
                        rinv[:, :, None].to_broadcast((P, KH, P)),
                        mybir.AluOpType.mult,
                    )
                    xdq = xdp.tile([P, KH, P], bf16, tag="xdq")
                    eng.tensor_tensor(
                        xdq[:], xq[:],
                        s2[:, :, None].to_broadcast((P, KH, P)),
                        mybir.AluOpType.mult,
                    )
                    xTh = xtp.tile([P, KH, P], bf16, tag="xT")
                    nc.sync.dma_start_transpose(
                        xTh[:], xdq[:].rearrange("p a b -> p (a b)")
                    )
                    halves.append(xTh)
                xts[mt] = halves

            pts = {}

            def half_job(mt, c, kh):
                # 16 matmuls: psum[mt,c] += x[mt, khalf].T @ w[chunk c, khalf]
                if kh == 0:
                    pts[(mt, c)] = psum.tile(
                        [P, CHW], mybir.dt.float32, name=f"pt{mt}_{c}", tag="pt"
                    )
                pt = pts[(mt, c)]
                xTh = xts[mt][kh]
                for kb in range(kh * KH, (kh + 1) * KH):
                    nc.tensor.matmul(
                        pt[:],
                        xTh[:, kb - kh * KH, :],
                        wTc[c][:, :, kb, :],
                        start=(kb == 0),
                        stop=(kb == KB - 1),
                    )

            def drain(mt, c):
                pt = pts.pop((mt, c))
                yt = ypool.tile([P, CHW], bf16, tag="yt")
                if c % 2 == 0:
                    nc.scalar.activation(
                        yt[:], pt[:], mybir.ActivationFunctionType.Copy
                    )
                else:
                    nc.vector.tensor_copy(yt[:], pt[:])
                nc.scalar.dma_start(
                    y_d[mt * P:(mt + 1) * P, c * CHW:(c + 1) * CHW], yt[:]
                )

            def chunk_job(mt, c):
                for kh in range(2):
                    half_job(mt, c, kh)
                drain(mt, c)

            # ---- emission ----
            NPREP = min(4, MT)
            x_prep(0)
            if MT > 1:
                x_prep(1)
            for kq in range(NQ):
                w_group(0, kq)
            if MT > 2:
                x_prep(2)
            if NCH > 1:
                for kq in range(NQ):
                    w_group(1, kq)
            if MT > 3:
                x_prep(3)

            # Ramp snake over the first 4 m-tiles: tiles join as their x
            # chain lands, chunks join as their weights land; late weight
            # chunks emitted just-in-time.
            prepped = NPREP
            SR = min(4, MT)
            if SR == 4 and NCH == 4:
                snake = [(0, 0), (1, 0), (0, 1), (1, 1),
                         (2, 0), (2, 1), (3, 0), (3, 1),
                         (0, 2), (1, 2), (2, 2), (3, 2),
                         (0, 3), (1, 3), (2, 3), (3, 3)]
            else:
                snake = [(mt, c) for mt in range(SR) for c in range(NCH)]
            for i, (mt, c) in enumerate(snake):
                if NCH > 2 and i == 1:
                    for kq in range(NQ):
                        w_group(2, kq)
                if NCH > 3 and i == 3:
                    for kq in range(NQ):
                        w_group(3, kq)
                chunk_job(mt, c)
            done = SR

            # Pair phase: chunk-major over (4,5) absorbs any weight tail.
            pair_mts = [mt for mt in (4, 5) if done <= mt < MT]
            if pair_mts:
                for mt in pair_mts:
                    if prepped <= mt:
                        x_prep(prepped)
                        prepped += 1
                for c in range(NCH):
                    for mt in pair_mts:
                        chunk_job(mt, c)
                done += len(pair_mts)

            # Steady state: m-tile-major.
            for mt in range(done, MT):
                while prepped < MT and prepped <= mt + 2:
                    x_prep(prepped)
                    prepped += 1
                for c in range(NCH):
                    chunk_job(mt, c)

    nc.compile()
    return nc


def kernel(x, weight, weight_scale_inv):
    from concourse.bass_utils import run_bass_kernel_spmd

    if "nc" not in _NC_CACHE:
        _NC_CACHE["nc"] = _build()
    nc = _NC_CACHE["nc"]

    x = np.ascontiguousarray(np.asarray(x, dtype=np.float32))
    weight = np.asarray(weight, dtype=np.float32)
    ws = np.asarray(weight_scale_inv, dtype=np.float32)

    in_maps = [
        {
            "x": x,
            "w": np.ascontiguousarray(weight[c * NSH:(c + 1) * NSH]),
            "ws": np.ascontiguousarray(ws[c * NB:(c + 1) * NB]),
        }
        for c in range(NCORES)
    ]
    res = run_bass_kernel_spmd(nc, in_maps, list(range(NCORES)))
    y = np.concatenate([res.results[c]["y"] for c in range(NCORES)], axis=1)
    return y.astype(np.float32, copy=False)
